# revision 8
# baseline (speedup 1.0000x reference)
"""Trainium2 Bass kernel for nn_CombinedLoss: weighted BCE (9x9 morphology
boundary weights) + soft dice, data-parallel over 8 NeuronCores.

Self-contained: hardcodes shapes [32,1,1024,1024] f32 and the sharding
(4 samples per core). Host combines tiny per-core partial sums.

V4 design notes (per core):
 - samples processed in pairs; Scalar ACT table switches (sigmoid <->
   ln sets) batched to 2 per pair.
 - per-pair schedule overlaps engines: z(s0), z(s1) on DVE; sigmoid/ln
   ACTs on Scalar run concurrently with the horizontal box-sum chains
   (cB double-buffered so chain(s1) can run while TensorE consumes
   chain(s0)).
 - z = x*(2t-1) megatile-wide; pads forced to +20 so they contribute
   exactly 1.0 to the sigmoid accum (subtracted host-side) and ~0 to
   the ln accum.
 - boundary mask combined into ONE test: boundary <=> 0.5 < S2D < tau
   <=> |S2D - c| < r. u = |S2D - c| on Scalar (ACT Abs, per-partition
   bias), then one stt(is_lt,mult) accumulates sum(bce * mask).
 - intersection I = sum(t*sgz): after the ln pass, sgz is multiplied
   in place by t (one megatile tt), then ones-matmul column sums
   accumulate in PSUM across each sample (TensorE). This also releases
   the targets tile early so the next pair's loads prefetch.
 - dice denominator: P + T = 2I - sum(sgz) + HW (target count
   cancels): no target-sum reduction needed.
 - mln reuses the zt pool buffers (zt dead once sigmoid ran).
"""

import numpy as np
import ml_dtypes

import concourse.bass as bass
import concourse.tile as tile
from concourse import bacc, mybir
from concourse.bass_utils import run_bass_kernel_spmd

AF = mybir.ActivationFunctionType
OP = mybir.AluOpType
BF16 = mybir.dt.bfloat16
F32 = mybir.dt.float32

B, H, W = 32, 1024, 1024
N_CORES = 8
SPC = B // N_CORES          # samples per core = 4
NB = H // 128               # 8 row blocks per sample
PAD = 8
SEG = W + 2 * PAD           # 1040 padded segment stride
MEGA = NB * SEG             # 8320
MEGAX = MEGA + 8            # +8 tail so shifted reads stay in-bounds
NSLOT = SPC * NB            # 32 accum slots per core
BOUNDARY_WEIGHT = 3.0
SMOOTH = 1.0
ZPAD = 20.0                 # zt pad value: sigmoid(20) == 1.0 in bf16

NPBF16 = ml_dtypes.bfloat16


def _cnt_v(r):
    return min(r, 4) + 1 + min(H - 1 - r, 4)


def _make_consts():
    k = np.arange(128)
    band9 = (np.abs(k[:, None] - k[None, :]) <= 4).astype(NPBF16)
    ht = np.zeros((128, 128), NPBF16)   # rows from PREV block (top halo)
    hb = np.zeros((128, 128), NPBF16)   # rows from NEXT block (bottom halo)
    for m in range(4):
        ht[124 + m:, m] = 1.0
    for m in range(124, 128):
        hb[: m - 123, m] = 1.0
    ones = np.ones((128, 1), NPBF16)
    # horizontal clipped-window rescale: data cols 0..3 and 1020..1023
    el = np.tile(np.array([9 / 5, 9 / 6, 9 / 7, 9 / 8], np.float32), NB)
    er = np.tile(np.array([9 / 8, 9 / 7, 9 / 6, 9 / 5], np.float32), NB)
    edgeL = np.broadcast_to(el, (128, NB * 4)).astype(NPBF16).copy()
    edgeR = np.broadcast_to(er, (128, NB * 4)).astype(NPBF16).copy()
    # combined-threshold constants: boundary <=> |S - c| < r
    cc = np.zeros((128, NB), np.float32)
    rr = np.zeros((128, NB), np.float32)
    for b in range(NB):
        for p in range(128):
            cv = 9.0 * _cnt_v(b * 128 + p)
            cc[p, b] = -cv / 2.0            # ACT bias = -c
            rr[p, b] = (cv - 1.0) / 2.0     # compare radius
    return {
        "band9": band9, "halo_t": ht, "halo_b": hb, "ones": ones,
        "edgeL": edgeL, "edgeR": edgeR, "cbias": cc, "rrad": rr,
    }


def _build_module():
    nc = bacc.Bacc("TRN2", target_bir_lowering=False, debug=False,
                   num_devices=N_CORES)

    lg = nc.dram_tensor("lg", [SPC, H, W], BF16, kind="ExternalInput").ap()
    tg = nc.dram_tensor("tg", [SPC, H, W], BF16, kind="ExternalInput").ap()
    band9 = nc.dram_tensor("band9", [128, 128], BF16, kind="ExternalInput").ap()
    halo_t = nc.dram_tensor("halo_t", [128, 128], BF16, kind="ExternalInput").ap()
    halo_b = nc.dram_tensor("halo_b", [128, 128], BF16, kind="ExternalInput").ap()
    onesd = nc.dram_tensor("ones", [128, 1], BF16, kind="ExternalInput").ap()
    edgeL = nc.dram_tensor("edgeL", [128, NB * 4], BF16, kind="ExternalInput").ap()
    edgeR = nc.dram_tensor("edgeR", [128, NB * 4], BF16, kind="ExternalInput").ap()
    cbiasd = nc.dram_tensor("cbias", [128, NB], F32, kind="ExternalInput").ap()
    rradd = nc.dram_tensor("rrad", [128, NB], F32, kind="ExternalInput").ap()

    o_bce = nc.dram_tensor("o_bce", [128, SPC], F32, kind="ExternalOutput").ap()
    o_sgz = nc.dram_tensor("o_sgz", [128, SPC], F32, kind="ExternalOutput").ap()
    o_gsum = nc.dram_tensor("o_gsum", [1, SPC * 512], F32,
                            kind="ExternalOutput").ap()
    o_isum = nc.dram_tensor("o_isum", [1, SPC * 512], F32,
                            kind="ExternalOutput").ap()

    with tile.TileContext(nc) as tc:
        with (
            tc.tile_pool(name="const", bufs=1) as cpool,
            tc.tile_pool(name="io", bufs=2) as iopool,
            tc.tile_pool(name="mega", bufs=1) as mpool,
            tc.tile_pool(name="cbp", bufs=2) as cbpool,
            tc.tile_pool(name="ztp", bufs=2) as ztpool,
            tc.tile_pool(name="sgp", bufs=2) as sgpool,
            tc.tile_pool(name="blk", bufs=2) as bpool,
            tc.tile_pool(name="acc", bufs=1) as apool,
            tc.tile_pool(name="ps", bufs=2, space="PSUM") as pspool,
            tc.tile_pool(name="psi", bufs=2, space="PSUM") as psipool,
        ):
            # ---- constants into SBUF
            band9_t = cpool.tile([128, 128], BF16, tag="band9")
            halo_t_t = cpool.tile([128, 128], BF16, tag="halo_t")
            halo_b_t = cpool.tile([128, 128], BF16, tag="halo_b")
            ones_t = cpool.tile([128, 1], BF16, tag="ones")
            edgeL_t = cpool.tile([128, NB * 4], BF16, tag="edgeL")
            edgeR_t = cpool.tile([128, NB * 4], BF16, tag="edgeR")
            cbias_t = cpool.tile([128, NB], F32, tag="cbias")
            rrad_t = cpool.tile([128, NB], F32, tag="rrad")
            nc.sync.dma_start(out=band9_t[:], in_=band9)
            nc.sync.dma_start(out=halo_t_t[:], in_=halo_t)
            nc.sync.dma_start(out=halo_b_t[:], in_=halo_b)
            nc.sync.dma_start(out=ones_t[:], in_=onesd)
            nc.sync.dma_start(out=edgeL_t[:], in_=edgeL)
            nc.sync.dma_start(out=edgeR_t[:], in_=edgeR)
            nc.sync.dma_start(out=cbias_t[:], in_=cbiasd)
            nc.sync.dma_start(out=rrad_t[:], in_=rradd)

            # ---- accumulator staging
            a_bce = apool.tile([128, SPC], F32, tag="a_bce")
            a_sgz = apool.tile([128, SPC], F32, tag="a_sgz")
            g_stage = apool.tile([1, SPC * 512], F32, tag="g_stage")
            i_stage = apool.tile([1, SPC * 512], F32, tag="i_stage")

            # ---- chain scratch (ping) — pong cB is double-buffered
            cA = mpool.tile([128, MEGAX], BF16, tag="cA")
            nc.vector.memset(cA[:, MEGA:MEGAX], 0.0)

            # Warm up both io buffers: pad columns are memset ONCE per
            # physical buffer (loads only write data columns, so pads
            # persist across buffer reuse and the per-pair DMA is not
            # serialized behind V-queue memsets).
            for _ in range(2):
                Lt = iopool.tile([128, MEGAX], BF16, tag="Lt")
                Tt = iopool.tile([128, MEGAX], BF16, tag="Tt")
                Tt3 = Tt[:, 0:MEGA].rearrange("p (b c) -> p b c", c=SEG)
                Lt3 = Lt[:, 0:MEGA].rearrange("p (b c) -> p b c", c=SEG)
                nc.vector.memset(Tt3[:, :, 0:PAD], 0.0)
                nc.vector.memset(Tt3[:, :, PAD + W:SEG], 0.0)
                nc.vector.memset(Tt[:, MEGA:MEGAX], 0.0)
                nc.vector.memset(Lt3[:, :, 0:PAD], 1.0)
                nc.vector.memset(Lt3[:, :, PAD + W:SEG], 1.0)

            def load_sample(s):
                Lt = iopool.tile([128, MEGAX], BF16, tag="Lt")
                Tt = iopool.tile([128, MEGAX], BF16, tag="Tt")
                Tt3 = Tt[:, 0:MEGA].rearrange("p (b c) -> p b c", c=SEG)
                Lt3 = Lt[:, 0:MEGA].rearrange("p (b c) -> p b c", c=SEG)
                nc.sync.dma_start(
                    out=Lt3[:, :, PAD:PAD + W],
                    in_=lg[s].rearrange("(b p) w -> p b w", p=128))
                nc.sync.dma_start(
                    out=Tt3[:, :, PAD:PAD + W],
                    in_=tg[s].rearrange("(b p) w -> p b w", p=128))
                return Lt, Tt

            def z_and_sig(s, Lt, Tt):
                # zt = (2t-1)*x megatile-wide; pads forced to +ZPAD
                zt = ztpool.tile([128, MEGA], BF16, tag="zt")
                zt3 = zt[:].rearrange("p (b c) -> p b c", c=SEG)
                nc.vector.tensor_scalar(
                    out=zt[:], in0=Tt[:, 0:MEGA],
                    scalar1=2.0, scalar2=-1.0, op0=OP.mult, op1=OP.add)
                nc.vector.memset(zt3[:, :, 0:PAD], ZPAD)
                nc.vector.memset(zt3[:, :, PAD + W:SEG], ZPAD)
                nc.vector.tensor_mul(zt[:], zt[:], Lt[:, 0:MEGA])
                sgz = sgpool.tile([128, MEGA], BF16, tag="sgz")
                nc.scalar.activation(sgz[:], zt[:], AF.Sigmoid,
                                     accum_out=a_sgz[:, s:s + 1])
                return sgz

            def ln_pass(s, sgz):
                mln = ztpool.tile([128, MEGA], BF16, tag="zt")  # reuse zt bufs
                nc.scalar.activation(mln[:], sgz[:], AF.Ln,
                                     accum_out=a_bce[:, s:s + 1])
                return mln

            def chain(Tt):
                # horizontal 9-box-sum (log chain), width MEGA
                cB = cbpool.tile([128, MEGAX], BF16, tag="cB")
                nc.vector.memset(cB[:, MEGA:MEGAX], 0.0)
                CW = MEGA
                nc.vector.tensor_add(cA[:, 0:CW], Tt[:, 0:CW], Tt[:, 1:CW + 1])
                nc.vector.tensor_add(cB[:, 0:CW], cA[:, 0:CW], cA[:, 2:CW + 2])
                nc.vector.tensor_add(cA[:, 0:CW], cB[:, 0:CW], cB[:, 4:CW + 4])
                nc.vector.tensor_add(cB[:, 0:CW], cA[:, 0:CW], Tt[:, 8:CW + 8])
                s9v = cB[:, 0:MEGA].rearrange("p (b c) -> p b c", c=SEG)
                eL3 = edgeL_t[:].rearrange("p (b c) -> p b c", c=4)
                eR3 = edgeR_t[:].rearrange("p (b c) -> p b c", c=4)
                nc.vector.tensor_mul(s9v[:, :, 4:8], s9v[:, :, 4:8], eL3)
                nc.vector.tensor_mul(s9v[:, :, 1024:1028],
                                     s9v[:, :, 1024:1028], eR3)
                return cB

            def q_inplace(Tt, sgz):
                # sgz *= t (megatile): sgz becomes the intersection product;
                # pads become 0 (t pads are 0). Releases Tt for prefetch.
                nc.vector.tensor_mul(sgz[:], Tt[:, 0:MEGA], sgz[:])

            def blocks(s, cB, sgz_q, mln):
                ipsum = psipool.tile([1, 512], F32, tag="ipsum")
                gpsum = psipool.tile([1, 512], F32, tag="gpsum")
                for b in range(NB):
                    o = b * SEG + PAD
                    slot = s * NB + b

                    # vertical 9-box-sum via banded matmuls into PSUM
                    S2D = pspool.tile([128, W], F32, tag="S2D")
                    mm = [(band9_t, b)]
                    if b > 0:
                        mm.append((halo_t_t, b - 1))
                    if b < NB - 1:
                        mm.append((halo_b_t, b + 1))
                    for i, (wt, bb) in enumerate(mm):
                        for h_ in range(2):
                            rc = bb * SEG + 4 + h_ * 512
                            nc.tensor.matmul(
                                S2D[:, h_ * 512:(h_ + 1) * 512],
                                wt[:], cB[:, rc:rc + 512],
                                start=(i == 0), stop=(i == len(mm) - 1))

                    # u = |S2D - c| on Scalar (PSUM -> SBUF bf16)
                    u = bpool.tile([128, W], BF16, tag="u")
                    nc.scalar.activation(u[:], S2D[:], AF.Abs,
                                         bias=cbias_t[:, b:b + 1], scale=1.0)

                    # masked bce: m = [u < r] (4x ts), p = m*mln (2x tt),
                    # ones-matmul column sums accumulate in PSUM
                    nc.vector.tensor_scalar(
                        out=u[:], in0=u[:], scalar1=rrad_t[:, b:b + 1],
                        scalar2=None, op0=OP.is_lt)
                    gq = bpool.tile([128, W], BF16, tag="junk")
                    nc.vector.tensor_mul(gq[:], u[:], mln[:, o:o + W])
                    nc.tensor.matmul(gpsum[:], ones_t[:], gq[:, 0:512],
                                     start=(b == 0), stop=False)
                    nc.tensor.matmul(gpsum[:], ones_t[:], gq[:, 512:1024],
                                     start=False, stop=(b == NB - 1))

                    # intersection: ones-matmul column sums of t*sgz
                    nc.tensor.matmul(ipsum[:], ones_t[:],
                                     sgz_q[:, o:o + 512],
                                     start=(b == 0), stop=False)
                    nc.tensor.matmul(ipsum[:], ones_t[:],
                                     sgz_q[:, o + 512:o + 1024],
                                     start=False, stop=(b == NB - 1))

                # stage I and G column sums (ACT Copy is in every set)
                nc.scalar.activation(
                    i_stage[0:1, s * 512:(s + 1) * 512], ipsum[0:1, :],
                    AF.Copy)
                nc.scalar.activation(
                    g_stage[0:1, s * 512:(s + 1) * 512], gpsum[0:1, :],
                    AF.Copy)

            # ---- paired-sample schedule
            for p in range(SPC // 2):
                s0, s1 = 2 * p, 2 * p + 1
                Lt0, Tt0 = load_sample(s0)
                Lt1, Tt1 = load_sample(s1)
                sgz0 = z_and_sig(s0, Lt0, Tt0)
                sgz1 = z_and_sig(s1, Lt1, Tt1)
                mln0 = ln_pass(s0, sgz0)
                mln1 = ln_pass(s1, sgz1)
                cB0 = chain(Tt0)
                q_inplace(Tt0, sgz0)
                cB1 = chain(Tt1)
                q_inplace(Tt1, sgz1)
                blocks(s0, cB0, sgz0, mln0)
                blocks(s1, cB1, sgz1, mln1)

            nc.sync.dma_start(out=o_bce, in_=a_bce[:])
            nc.sync.dma_start(out=o_sgz, in_=a_sgz[:])
            nc.sync.dma_start(out=o_gsum, in_=g_stage[:])
            nc.sync.dma_start(out=o_isum, in_=i_stage[:])
    nc.finalize()
    return nc


_NC = None


def _get_module():
    global _NC
    if _NC is None:
        _NC = _build_module()
    return _NC


def _run(logits, targets, trace=False):
    lg = np.ascontiguousarray(
        np.asarray(logits, np.float32).reshape(B, H, W).astype(NPBF16))
    tg = np.ascontiguousarray(
        np.asarray(targets, np.float32).reshape(B, H, W).astype(NPBF16))
    consts = _make_consts()
    nc = _get_module()
    in_maps = []
    for c in range(N_CORES):
        m = dict(consts)
        m["lg"] = lg[c * SPC:(c + 1) * SPC]
        m["tg"] = tg[c * SPC:(c + 1) * SPC]
        in_maps.append(m)
    res = run_bass_kernel_spmd(nc, in_maps, core_ids=list(range(N_CORES)),
                               trace=trace)
    return res


def _combine(results):
    HW = H * W
    wb = 0.0
    scores = []
    for c in range(N_CORES):
        r = results[c]
        # o_bce holds sum of ln(sigmoid(z)) = -bce (pads add ~ln(1)=0)
        # o_g holds sum of ln(sigmoid(z)) * boundary_mask
        bce_s = -r["o_bce"].astype(np.float64).sum()
        gm = -r["o_gsum"].astype(np.float64).sum()
        wb += bce_s + (BOUNDARY_WEIGHT - 1.0) * gm
        for s in range(SPC):
            I = r["o_isum"][0, s * 512:(s + 1) * 512].astype(np.float64).sum()
            # sigmoid accum includes 128 pad cols of exactly 1.0 per row
            sgz_sum = r["o_sgz"][:, s].astype(np.float64).sum() - 128.0 * 128.0
            # P + T = 2I - sum(sgz) + HW  (target count cancels)
            scores.append(2.0 * (I + SMOOTH) / (2.0 * I - sgz_sum + HW + SMOOTH))
    bce = wb / (B * HW)
    dice = 1.0 - np.mean(scores)
    return np.float32(bce + dice)


def kernel(logits, targets):
    res = _run(logits, targets, trace=False)
    return _combine(res.results)
